# revision 1
# baseline (speedup 1.0000x reference)
"""Cross-attention kernel for Trainium2, sharded over 8 NeuronCores.

Sharding: rows of (B, S1) split 8 ways -> each core handles one batch's
half (2048 query rows) and recomputes that batch's small KV projection.
No collectives needed.

Host-side preprocessing (free - doesn't count toward HW time):
  - transpose x, y to feature-major, pad with a ones-row so the PE adds bq
  - transpose weights; per-head deinterleave permutation of the head_dim
    axis on the Q/K sides turns the reference's interleaved rotate_half
    into contiguous-half rotation
  - fold qn_w, kn_w and the attention scale into cos/sin tables / LN rstd
  - kn_b is dropped entirely: it shifts all scores of a row equally, which
    softmax cancels.

Device pipeline per 128-row chunk (all matmuls bf16, fp32 accumulation):
  Q-proj (PE) -> per-head LN stats (bn_stats) + apply (tensor_scalar)
  -> RoPE (gpsimd + DVE, cos/sin prefolded) -> DMA-transpose q per head
  -> scores (PE) -> exp with free denominator (ACT accum_out)
  -> normalize (ACT copy w/ per-partition scale) -> DMA-transpose attn
  -> PV (PE, feature-major ctx) -> out-proj (PE) -> +bout (DVE) -> DMA out.
"""
import sys

sys.path.insert(0, '/opt/trn_rl_repo')

import numpy as np
import ml_dtypes

import concourse.bass as bass
import concourse.tile as tile
from concourse import bacc, mybir
from concourse.bass_utils import run_bass_kernel_spmd

bf16 = mybir.dt.bfloat16
f32 = mybir.dt.float32

# problem shapes (hardcoded per contest rules)
B, S1, S2, CQ, CKV, H, D = 4, 4096, 256, 1408, 1024, 16, 88
NCORES = 8
S = (B * S1) // NCORES          # 2048 query rows per core
NS = S // 128                   # 16 s-chunks
DP = 128                        # head_dim padded for transposes
G = 4                           # heads per LN group (4*88 = 352 <= 512 psum)
NG = H // G
KC_Q = CQ // 128 + 1            # 12 contraction chunks (incl. bias ones-row)
KC_KV = CKV // 128              # 8
KC_O = CQ // 128                # 11
EPS = 1e-6
HALF = D // 2                   # 44

_BUILD_CACHE = {}


def _build(use_badd: bool, reps: int = 1):
    nc = bacc.Bacc("TRN2", target_bir_lowering=False)

    xT = nc.dram_tensor("xT", [128 * KC_Q, S], bf16, kind="ExternalInput")
    yT = nc.dram_tensor("yT", [CKV, S2], bf16, kind="ExternalInput")
    wq = nc.dram_tensor("wq", [128 * KC_Q, CQ], bf16, kind="ExternalInput")
    wkv = nc.dram_tensor("wkv", [CKV, 2 * CQ], bf16, kind="ExternalInput")
    wout = nc.dram_tensor("wout", [CQ, CQ], bf16, kind="ExternalInput")
    bkv = nc.dram_tensor("bkv", [2 * CQ], bf16, kind="ExternalInput")
    bout = nc.dram_tensor("bout", [CQ], bf16, kind="ExternalInput")
    cw = nc.dram_tensor("cw", [S, DP], f32, kind="ExternalInput")
    sw = nc.dram_tensor("sw", [S, DP], f32, kind="ExternalInput")
    if use_badd:
        badd = nc.dram_tensor("badd", [S, DP], f32, kind="ExternalInput")
    out = nc.dram_tensor("out", [S, CQ], f32, kind="ExternalOutput")

    # kv projection output tiling: 4 k-groups of 352, then v in 512/512/384
    k_tiles = [(g * 352, 352) for g in range(NG)]
    v_tiles = [(CQ, 512), (CQ + 512, 512), (CQ + 1024, 384)]
    o_tiles = [(0, 512), (512, 512), (1024, 384)]

    with tile.TileContext(nc) as tc:
        with (
            tc.tile_pool(name="persist", bufs=1) as persist,
            tc.tile_pool(name="xq", bufs=3) as xqp,
            tc.tile_pool(name="cs", bufs=4) as csp,
            tc.tile_pool(name="qwork", bufs=2) as qwork,
            tc.tile_pool(name="stats", bufs=4) as statsp,
            tc.tile_pool(name="qrope", bufs=2) as qropep,
            tc.tile_pool(name="qropeT", bufs=3) as qropeTp,
            tc.tile_pool(name="attn", bufs=3) as attnp,
            tc.tile_pool(name="attnT", bufs=2) as attnTp,
            tc.tile_pool(name="cbf", bufs=4) as cbfp,
            tc.tile_pool(name="ctxT", bufs=2) as ctxTp,
            tc.tile_pool(name="outsb", bufs=2) as outsbp,
            tc.tile_pool(name="ps_big", bufs=2, space="PSUM") as ps_big,
            tc.tile_pool(name="ps_o", bufs=2, space="PSUM") as ps_o,
            tc.tile_pool(name="ps_sc", bufs=2, space="PSUM") as ps_sc,
            tc.tile_pool(name="ps_ctx", bufs=2, space="PSUM") as ps_ctx,
        ):
            # ---------- persistent tiles ----------
            wq_sb = persist.tile([128, KC_Q, CQ], bf16, tag="wq_sb")
            for _g in range(NG):
                nc.sync.dma_start(
                    wq_sb[:, :, _g * 352:(_g + 1) * 352],
                    wq[:].rearrange("(k p) o -> p k o", p=128)
                    [:, :, _g * 352:(_g + 1) * 352])
            wout_sb = persist.tile([128, KC_O, CQ], bf16, tag="wout_sb")
            nc.sync.dma_start(wout_sb[:], wout[:].rearrange("(k p) o -> p k o", p=128))
            yT_sb = persist.tile([128, KC_KV, S2], bf16, tag="yT_sb")
            nc.sync.dma_start(yT_sb[:], yT[:].rearrange("(k p) t -> p k t", p=128))

            bkv_ap = bkv[:]
            bkv_bc = persist.tile([128, 2 * CQ], bf16, tag="bkv_bc")
            nc.gpsimd.dma_start(bkv_bc[:], bass.AP(
                tensor=bkv_ap.tensor, offset=bkv_ap.offset,
                ap=[[0, 128], *bkv_ap.ap]))
            bout_ap = bout[:]
            bout_bc = persist.tile([128, CQ], bf16, tag="bout_bc")
            nc.gpsimd.dma_start(bout_bc[:], bass.AP(
                tensor=bout_ap.tensor, offset=bout_ap.offset,
                ap=[[0, 128], *bout_ap.ap]))

            def emit_rsqrt(y, v_ap, n, post_scale=None):
                # y = 1/sqrt(v + EPS) via Newton iterations (all DVE, tiny)
                nc.vector.tensor_scalar(
                    out=y[:], in0=v_ap, scalar1=-0.5, scalar2=1.5 + EPS,
                    op0=mybir.AluOpType.mult, op1=mybir.AluOpType.add)
                nc.vector.tensor_scalar_max(out=y[:], in0=y[:], scalar1=0.08)
                t1 = statsp.tile([128, n], f32, tag="nr_t1")
                for _ in range(3):
                    nc.vector.tensor_mul(t1[:], y[:], y[:])
                    nc.vector.tensor_tensor(t1[:], t1[:], v_ap,
                                            mybir.AluOpType.mult)
                    nc.vector.tensor_scalar(
                        out=t1[:], in0=t1[:], scalar1=-0.5, scalar2=1.5 + 0.5 * EPS,
                        op0=mybir.AluOpType.mult, op1=mybir.AluOpType.add)
                    nc.vector.tensor_mul(y[:], y[:], t1[:])
                if post_scale is not None:
                    nc.vector.tensor_scalar_mul(out=y[:], in0=y[:],
                                                scalar1=post_scale)

            kln = [persist.tile([128, H, DP], bf16, tag=f"kln{t}", name=f"kln{t}")
                   for t in range(2)]
            # kT layout: [d_pad, head, t]
            kT = persist.tile([128, H, S2], bf16, tag="kT")
            v_sb = persist.tile([128, 2, CQ], bf16, tag="v_sb")

            def emit_body():
                # ---------- KV phase ----------
                for t in range(2):
                    nc.gpsimd.memset(kln[t][:, :, D:DP], 0.0)
                for (o0, ow) in k_tiles + v_tiles:
                    wkv_t = attnp.tile([128, KC_KV, ow], bf16, tag="attn")
                    nc.sync.dma_start(
                        wkv_t[:],
                        wkv[:].rearrange("(k p) o -> p k o", p=128)[:, :, o0:o0 + ow])
                    for t in range(2):
                        ps = ps_o.tile([128, 512], f32, tag="pso")
                        for kc in range(KC_KV):
                            nc.tensor.matmul(
                                ps[:, :ow],
                                yT_sb[:, kc, t * 128:(t + 1) * 128],
                                wkv_t[:, kc, :],
                                start=(kc == 0), stop=(kc == KC_KV - 1))
                        if o0 < CQ:
                            g0 = o0 // 352 * G
                            kb = qwork.tile([128, G, D], f32, tag="kb")
                            nc.vector.tensor_tensor(
                                kb[:].rearrange("p g d -> p (g d)"), ps[:, :ow],
                                bkv_bc[:, o0:o0 + ow], mybir.AluOpType.add)
                            st = statsp.tile([128, G, 6], f32, tag="st")
                            mv = statsp.tile([128, G, 2], f32, tag="mv")
                            for g in range(G):
                                nc.vector.bn_stats(st[:, g, :], kb[:, g, :])
                                nc.vector.bn_aggr(mv[:, g, :], st[:, g, :])
                            rstd = statsp.tile([128, G], f32, tag="rstd")
                            emit_rsqrt(rstd, mv[:, :, 1], G)
                            for g in range(G):
                                nc.vector.tensor_scalar(
                                    out=kln[t][:, g0 + g, 0:D], in0=kb[:, g, :],
                                    scalar1=mv[:, g, 0:1], scalar2=rstd[:, g:g + 1],
                                    op0=mybir.AluOpType.subtract,
                                    op1=mybir.AluOpType.mult)
                        else:
                            nc.vector.tensor_tensor(
                                v_sb[:, t, o0 - CQ:o0 - CQ + ow], ps[:, :ow],
                                bkv_bc[:, CQ + (o0 - CQ):CQ + (o0 - CQ) + ow],
                                mybir.AluOpType.add)
                for t in range(2):
                    nc.sync.dma_start_transpose(
                        kT[:, :, t * 128:(t + 1) * 128],
                        kln[t][:].rearrange("p h d -> p (h d)"))

                # ---------- main loop over s-chunks ----------
                aT_tiles = [None, None]
                for si in range(NS):
                    xq = xqp.tile([128, KC_Q, 128], bf16, tag="xq")
                    nc.scalar.dma_start(
                        xq[:], xT[:].rearrange("(k p) s -> p k s", p=128)
                        [:, :, si * 128:(si + 1) * 128])
                    cw_sb = csp.tile([128, DP], f32, tag="cs")
                    nc.scalar.dma_start(cw_sb[:], cw[si * 128:(si + 1) * 128, :])
                    sw_sb = csp.tile([128, DP], f32, tag="cs")
                    nc.scalar.dma_start(sw_sb[:], sw[si * 128:(si + 1) * 128, :])
                    if use_badd:
                        ba_sb = csp.tile([128, DP], f32, tag="cs")
                        nc.scalar.dma_start(ba_sb[:], badd[si * 128:(si + 1) * 128, :])

                    qrope = qropep.tile([128, H, DP], bf16, tag="qrope")
                    nc.gpsimd.memset(qrope[:, :, D:DP], 0.0)
                    qropeT = qropeTp.tile([128, H, 128], bf16, tag="qropeT")

                    mv_all = statsp.tile([128, H, 2], f32, tag="mv_all")
                    for g in range(NG):
                        ps = ps_big.tile([128, 512], f32, tag="big")
                        for kc in range(KC_Q):
                            nc.tensor.matmul(
                                ps[:, :352],
                                xq[:, kc, :],
                                wq_sb[:, kc, g * 352:(g + 1) * 352],
                                start=(kc == 0), stop=(kc == KC_Q - 1))
                        psv = ps[:, :352].rearrange("p (g d) -> p g d", d=D)
                        st = statsp.tile([128, G, 6], f32, tag="st")
                        for g2 in range(G):
                            nc.vector.bn_stats(st[:, g2, :], psv[:, g2, :])
                            nc.vector.bn_aggr(mv_all[:, g * G + g2, :], st[:, g2, :])
                        qcr = qwork.tile([128, G, D], f32, tag="qcr")
                        for g2 in range(G):
                            nc.vector.tensor_scalar_sub(
                                out=qcr[:, g2, :], in0=psv[:, g2, :],
                                scalar1=mv_all[:, g * G + g2, 0:1])
                        tt = qwork.tile([128, G, D], f32, tag="tt")
                        nc.gpsimd.tensor_mul(
                            tt[:, :, 0:HALF], qcr[:, :, HALF:D],
                            sw_sb[:, None, 0:HALF].to_broadcast([128, G, HALF]))
                        nc.gpsimd.tensor_mul(
                            tt[:, :, HALF:D], qcr[:, :, 0:HALF],
                            sw_sb[:, None, HALF:D].to_broadcast([128, G, HALF]))
                        u = qwork.tile([128, G, D], f32, tag="u")
                        nc.vector.tensor_mul(
                            u[:], qcr[:],
                            cw_sb[:, None, 0:D].to_broadcast([128, G, D]))
                        if use_badd:
                            nc.vector.tensor_add(u[:], u[:], ba_sb[:, None, 0:D]
                                                 .to_broadcast([128, G, D]))
                        nc.vector.tensor_add(qrope[:, g * G:(g + 1) * G, 0:D],
                                             u[:], tt[:])
                        if g % 2 == 1:
                            h0 = (g - 1) * G
                            nc.sync.dma_start_transpose(
                                qropeT[:, h0:h0 + 2 * G, :],
                                qrope[:, h0:h0 + 2 * G, :]
                                .rearrange("p h d -> p (h d)"))

                    # rstd for all heads via Newton (DVE only), fold D^-1/2
                    rstd_all = statsp.tile([128, H], f32, tag="rstd_all")
                    emit_rsqrt(rstd_all, mv_all[:, :, 1], H,
                               post_scale=float(D) ** -0.5)


                    # scores + softmax (row-major [s, t])
                    denom = statsp.tile([128, H], f32, tag="denom")
                    attn = attnp.tile([128, H, S2], bf16, tag="attn")
                    for h in range(H):
                        sps = ps_sc.tile([128, S2], f32, tag="sc")
                        nc.tensor.matmul(sps[:], qropeT[:, h, :], kT[:, h, :],
                                         start=True, stop=True)
                        nc.scalar.activation(
                            out=attn[:, h, :], in_=sps[:],
                            func=mybir.ActivationFunctionType.Exp,
                            scale=rstd_all[:, h:h + 1],
                            accum_out=denom[:, h:h + 1])
                    rd = statsp.tile([128, H], f32, tag="rd")
                    nc.vector.reciprocal(rd[:], denom[:])
                    aT = attnTp.tile([128, 2 * H, 128], bf16, tag="attnT")
                    aT_tiles[si % 2] = aT
                    for g in range(NG):
                        for h in range(g * G, (g + 1) * G):
                            nc.gpsimd.tensor_scalar_mul(
                                out=attn[:, h, :], in0=attn[:, h, :],
                                scalar1=rd[:, h:h + 1])
                        if g % 2 == 1:
                            h0 = (g - 1) * G
                            nc.sync.dma_start_transpose(
                                aT[:, 2 * h0:2 * h0 + 4 * G, :],
                                attn[:, h0:h0 + 2 * G, :]
                                .rearrange("p h t -> p (h t)"))

                    # every 2 s-chunks: PV, ctx evac, out-proj
                    if si % 2 == 1:
                        ctxT = ctxTp.tile([128, KC_O, 256], bf16, tag="ctxT")
                        dma_engines = [nc.sync, nc.scalar, nc.gpsimd]
                        for h in range(H):
                            cps = ps_ctx.tile([D, 256], f32, tag="cps")
                            nmm = 0
                            for s2 in range(2):
                                for t in range(2):
                                    nc.tensor.matmul(
                                        cps[:, s2 * 128:(s2 + 1) * 128],
                                        v_sb[:, t, h * D:(h + 1) * D],
                                        aT_tiles[s2][:, 2 * h + t, :],
                                        start=(t == 0), stop=(t == 1))
                                    nmm += 1
                            cbf = cbfp.tile([D, 256], bf16, tag="cbf")
                            if h % 2 == 0:
                                nc.vector.tensor_copy(cbf[:], cps[:])
                            else:
                                nc.scalar.copy(cbf[:], cps[:])
                            c0 = h * D
                            r0, ch0 = c0 % 128, c0 // 128
                            n1 = min(128 - r0, D)
                            eng = dma_engines[h % 2]
                            eng.dma_start(ctxT[r0:r0 + n1, ch0, :], cbf[0:n1, :])
                            if n1 < D:
                                eng.dma_start(ctxT[0:D - n1, ch0 + 1, :],
                                              cbf[n1:D, :])

                        for s2 in range(2):
                            sj = si - 1 + s2
                            for (o0, ow) in o_tiles:
                                pso = ps_o.tile([128, 512], f32, tag="pso")
                                for c in range(KC_O):
                                    nc.tensor.matmul(
                                        pso[:, :ow],
                                        ctxT[:, c, s2 * 128:(s2 + 1) * 128],
                                        wout_sb[:, c, o0:o0 + ow],
                                        start=(c == 0), stop=(c == KC_O - 1))
                                osb = outsbp.tile([128, 512], f32, tag="outsb")
                                nc.vector.tensor_tensor(
                                    osb[:, :ow], pso[:, :ow],
                                    bout_bc[:, o0:o0 + ow], mybir.AluOpType.add)
                                nc.sync.dma_start(
                                    out[sj * 128:(sj + 1) * 128, o0:o0 + ow],
                                    osb[:, :ow])

            for _rep in range(reps):
                emit_body()

    nc.finalize()
    return nc


def _prep(inputs):
    """Host-side shared (per-core independent parts built in kernel())."""
    x = np.asarray(inputs['x'], np.float32)
    y = np.asarray(inputs['y'], np.float32)
    cos = np.asarray(inputs['cos'], np.float32)
    sin = np.asarray(inputs['sin'], np.float32)
    Wq = np.asarray(inputs['Wq'], np.float32)
    bq = np.asarray(inputs['bq'], np.float32)
    Wkv = np.asarray(inputs['Wkv'], np.float32)
    bkv = np.asarray(inputs['bkv'], np.float32)
    qn_w = np.asarray(inputs['qn_w'], np.float32)
    qn_b = np.asarray(inputs['qn_b'], np.float32)
    kn_w = np.asarray(inputs['kn_w'], np.float32)
    kn_b = np.asarray(inputs['kn_b'], np.float32)  # noqa: F841  (cancels in softmax)
    Wout = np.asarray(inputs['Wout'], np.float32)
    bout = np.asarray(inputs['bout'], np.float32)

    perm = np.concatenate([np.arange(0, D, 2), np.arange(1, D, 2)])
    swapv = np.concatenate([np.arange(HALF, D), np.arange(0, HALF)])
    sign = np.concatenate([-np.ones(HALF, np.float32), np.ones(HALF, np.float32)])

    # Q weights: permute head_dim within each head, transpose, append bias row
    Wq_p = Wq.reshape(H, D, CQ)[:, perm, :].reshape(CQ, CQ)
    bq_p = bq.reshape(H, D)[:, perm].reshape(CQ)
    wq_ext = np.zeros((128 * KC_Q, CQ), np.float32)
    wq_ext[:CQ] = Wq_p.T
    wq_ext[CQ] = bq_p

    # KV: permute k-half head_dim (bias too), transpose
    Wkv_p = Wkv.reshape(2, H, D, CKV).copy()
    Wkv_p[0] = Wkv_p[0][:, perm, :]
    bkv_p = bkv.reshape(2, H, D).copy()
    bkv_p[0] = bkv_p[0][:, perm]
    wkvT = Wkv_p.reshape(2 * CQ, CKV).T.copy()
    bkv_p = bkv_p.reshape(2 * CQ)

    wq_vec = qn_w[perm]
    wk_vec = kn_w[perm]
    bq_ln = qn_b[perm]

    cos_p = cos[:, perm]
    sin_p = sin[:, perm]
    wfold = wq_vec * wk_vec
    CW = cos_p * wfold[None, :]                                   # [S1, D]
    SW = sign[None, :] * sin_p * (wq_vec[swapv] * wk_vec)[None, :]
    use_badd = bool(np.any(bq_ln != 0.0))
    BA = wk_vec[None, :] * (bq_ln[None, :] * cos_p
                            + sign[None, :] * bq_ln[swapv][None, :] * sin_p)

    return dict(
        x=x, y=y, wq_ext=wq_ext, wkvT=wkvT, bkv_p=bkv_p,
        woutT=Wout.T.copy(), bout=bout, CW=CW, SW=SW, BA=BA,
        use_badd=use_badd)


def _make_in_maps(p):
    use_badd = p['use_badd']
    wq_bf = p['wq_ext'].astype(ml_dtypes.bfloat16)
    wkv_bf = p['wkvT'].astype(ml_dtypes.bfloat16)
    wout_bf = p['woutT'].astype(ml_dtypes.bfloat16)
    in_maps = []
    for c in range(NCORES):
        b = c // 2
        s0 = (c % 2) * S
        xTe = np.zeros((128 * KC_Q, S), np.float32)
        xTe[:CQ] = p['x'][b, s0:s0 + S].T
        xTe[CQ] = 1.0
        cwp = np.zeros((S, DP), np.float32)
        cwp[:, :D] = p['CW'][s0:s0 + S]
        swp = np.zeros((S, DP), np.float32)
        swp[:, :D] = p['SW'][s0:s0 + S]
        m = {
            'xT': xTe.astype(ml_dtypes.bfloat16),
            'yT': p['y'][b].T.astype(ml_dtypes.bfloat16).copy(),
            'wq': wq_bf, 'wkv': wkv_bf, 'wout': wout_bf,
            'bkv': p['bkv_p'].astype(ml_dtypes.bfloat16),
            'bout': p['bout'].astype(ml_dtypes.bfloat16),
            'cw': cwp, 'sw': swp,
        }
        if use_badd:
            bap = np.zeros((S, DP), np.float32)
            bap[:, :D] = p['BA'][s0:s0 + S]
            m['badd'] = bap
        in_maps.append(m)
    return in_maps


def get_nc(use_badd, reps=1):
    key = (use_badd, reps)
    if key not in _BUILD_CACHE:
        _BUILD_CACHE[key] = _build(use_badd, reps)
    return _BUILD_CACHE[key]


def kernel(**inputs) -> np.ndarray:
    p = _prep(inputs)
    in_maps = _make_in_maps(p)
    nc = get_nc(p['use_badd'])
    res = run_bass_kernel_spmd(nc, in_maps, core_ids=list(range(NCORES)))
    outp = np.empty((B, S1, CQ), np.float32)
    for c in range(NCORES):
        b = c // 2
        s0 = (c % 2) * S
        outp[b, s0:s0 + S] = res.results[c]['out']
    return outp



# revision 41
# speedup vs baseline: 1.6990x; 1.6990x over previous
"""Cross-attention kernel for Trainium2, sharded over 8 NeuronCores.

Sharding: rows of (B, S1) split 8 ways -> each core handles one batch's
half (2048 query rows) and recomputes that batch's small KV projection.
No collectives needed.

Host-side preprocessing (free - doesn't count toward HW time):
  - transpose x, y to feature-major, pad with a ones-row so the PE adds bq
  - transpose weights; per-head deinterleave permutation of the head_dim
    axis turns the interleaved rotate_half into contiguous-half rotation
  - LayerNorm mean-centering is linear -> folded into Wq/bq and the K half
    of Wkv/bkv on the host (W' = (I - 11^T/D) W per head). The device only
    needs the second moment for rstd.
  - qn_w, kn_w and the attention scale D^-1/2 folded into cos/sin tables;
    kn_b dropped (softmax-cancels).

Device pipeline, software-pipelined across s-chunks of 128 rows:
  Q-proj (PE, 3x512-wide psum slices) with the PREVIOUS chunk's scores
  matmuls interleaved between contraction passes so the PE never idles
  -> variance: Square (ACT) + windowed tensor_reduce (DVE)
  -> rstd via Newton rsqrt (DVE; keeps a single activation table)
  -> RoPE as whole-chunk broadcast ops writing qrope in place (DVE)
  -> DMA-transpose q -> scores (PE) -> exp w/ accumulated denom (ACT)
  -> normalize (tensor_scalar, split DVE/Pool) -> DMA-transpose attn into
     paired [t, 2h+tc, 256] tiles -> PV at N=256 (PE) -> ctx stays
     head-major in SBUF (no repack DMAs; out-proj contracts 16 per-head
     88-partition chunks) -> out-proj with bias via K=1 ones-row matmuls.
  KV projection (mean-centered K + LN via same square/reduce path) is
  folded into chunk 0's pipeline so the PE computes while weights stream.
"""
import sys

sys.path.insert(0, '/opt/trn_rl_repo')

import numpy as np
import ml_dtypes

import concourse.bass as bass
import concourse.tile as tile
from concourse import bacc, mybir
from concourse.bass_utils import run_bass_kernel_spmd

bf16 = mybir.dt.bfloat16
f32 = mybir.dt.float32
AF = mybir.ActivationFunctionType
ALU = mybir.AluOpType

# problem shapes (hardcoded per contest rules)
B, S1, S2, CQ, CKV, H, D = 4, 4096, 256, 1408, 1024, 16, 88
NCORES = 8
S = (B * S1) // NCORES          # 2048 query rows per core
NS = S // 128                   # 16 s-chunks
DP = 128                        # cw/sw table padding (host side)
DT = 128                        # head_dim pad for q/k transposes (the Ant
                                # transpose works in fixed 128-col tiles)
KC_Q = CQ // 128 + 1            # 12 contraction chunks (incl. bias ones-row)
KC_KV = CKV // 128              # 8
KC_O = CQ // 128                # 11
EPS = 1e-6
HALF = D // 2                   # 44
OSL = [(0, 512), (512, 512), (1024, 384)]   # 1408 = 512+512+384
# consumer split: pass A = psum slices 0,1 (features 0..1023) covers heads
# 0..10 fully (features 0..967); pass B completes heads 11..15
HA, HB = 11, 5
FA = HA * D                     # 968

_BUILD_CACHE = {}


def _build(use_badd: bool, reps: int = 1):
    nc = bacc.Bacc("TRN2", target_bir_lowering=False)

    xT = nc.dram_tensor("xT", [128 * KC_Q, S], bf16, kind="ExternalInput")
    yT = nc.dram_tensor("yT", [CKV, S2], bf16, kind="ExternalInput")
    wq = nc.dram_tensor("wq", [128 * KC_Q, CQ], bf16, kind="ExternalInput")
    wkv = nc.dram_tensor("wkv", [CKV, 2 * CQ], bf16, kind="ExternalInput")
    wout = nc.dram_tensor("wout", [CQ, CQ], bf16, kind="ExternalInput")
    bkv = nc.dram_tensor("bkv", [2 * CQ], bf16, kind="ExternalInput")
    bout = nc.dram_tensor("bout", [CQ], bf16, kind="ExternalInput")
    cw = nc.dram_tensor("cw", [S, DP], f32, kind="ExternalInput")
    sw = nc.dram_tensor("sw", [S, DP], f32, kind="ExternalInput")
    if use_badd:
        badd = nc.dram_tensor("badd", [S, DP], f32, kind="ExternalInput")
    out = nc.dram_tensor("out", [S, CQ], bf16, kind="ExternalOutput")

    # kv projection output tiling: 4 k-groups of 352, then v in 512/512/384
    k_tiles = [(g * 352, 352) for g in range(4)]
    v_tiles = [(CQ, 512), (CQ + 512, 512), (CQ + 1024, 384)]

    with tile.TileContext(nc) as tc:
        with (
            tc.tile_pool(name="persist", bufs=1) as persist,
            tc.tile_pool(name="xq", bufs=2) as xqp,
            tc.tile_pool(name="cs", bufs=2) as csp,
            tc.tile_pool(name="sq1", bufs=1) as sqp1,
            tc.tile_pool(name="sq", bufs=2) as sqp,
            tc.tile_pool(name="stats", bufs=4) as statsp,
            tc.tile_pool(name="qropeT", bufs=2) as qropeTp,
            tc.tile_pool(name="attn", bufs=2) as attnp,
            tc.tile_pool(name="cstage", bufs=2) as cstagep,
            tc.tile_pool(name="outsb", bufs=2) as outsbp,
            tc.tile_pool(name="outsb", bufs=2) as outsbp,
            tc.tile_pool(name="ps_q", bufs=1, space="PSUM") as ps_q,
            tc.tile_pool(name="ps_sc", bufs=2, space="PSUM") as ps_sc,
            tc.tile_pool(name="ps_ctx", bufs=1, space="PSUM") as ps_ctx,
            tc.tile_pool(name="ps_o", bufs=2, space="PSUM") as ps_o,
        ):
            # ---------- persistent tiles / initial DMAs ----------
            # order matters: KV-phase inputs first so the PE can start early;
            # the big Q/out weights stream in behind them on other queues.
            yT_sb = persist.tile([128, KC_KV, S2], bf16, tag="yT_sb")
            nc.sync.dma_start(yT_sb[:], yT[:].rearrange("(k p) t -> p k t", p=128))

            bkv_bc = persist.tile([1, 2 * CQ], bf16, tag="bkv_bc")
            nc.gpsimd.dma_start(bkv_bc[0:1, :], bkv[:][None, :])
            bout_bc = persist.tile([1, CQ], bf16, tag="bout_bc")
            nc.gpsimd.dma_start(bout_bc[0:1, :], bout[:][None, :])

            wq_sb = persist.tile([128, KC_Q, CQ], bf16, tag="wq_sb")
            for _g in range(3):
                nc.gpsimd.dma_start(
                    wq_sb[:, _g * 4:(_g + 1) * 4, :],
                    wq[:].rearrange("(k p) o -> p k o", p=128)
                    [:, _g * 4:(_g + 1) * 4, :])
            wout_sb = persist.tile([D, H, CQ], bf16, tag="wout_sb")

            def emit_wout_load():
                for _g in range(4):
                    nc.gpsimd.dma_start(
                        wout_sb[:, :, _g * 352:(_g + 1) * 352],
                        wout[:].rearrange("(h d) o -> d h o", d=D)
                        [:, :, _g * 352:(_g + 1) * 352])

            ones1 = persist.tile([1, 128], bf16, tag="ones1")
            nc.gpsimd.memset(ones1[:], 1.0)
            kT = persist.tile([DT, H, S2], bf16, tag="kT")
            v_sb = persist.tile([128, 2, CQ], bf16, tag="v_sb")
            # paired attn-transpose tiles: pair k -> aTp[k % 2]
            aTp = [persist.tile([128, 2 * H, 256], bf16, tag=f"aTp{t}", name=f"aTp{t}")
                   for t in range(2)]
            # double-buffered qrope with pad cols zeroed once; the KV phase
            # borrows these as its k-LN staging tiles (same shape, same pad)
            qrope2 = [persist.tile([128, H, DT], bf16, tag=f"qrope{t}", name=f"qrope{t}")
                      for t in range(2)]
            for t in range(2):
                nc.gpsimd.memset(qrope2[t][:, :, D:DT], 0.0)

            def emit_rstd(rstd, ssq_ap, n, scale):
                # v = ssq*scale + EPS; rstd = v**-0.5 via Newton (all DVE)
                v = statsp.tile([128, n], f32, tag="nv")
                nc.vector.tensor_scalar(
                    out=v[:], in0=ssq_ap, scalar1=float(scale), scalar2=EPS,
                    op0=ALU.mult, op1=ALU.add)
                nc.vector.tensor_scalar(
                    out=rstd[:], in0=v[:], scalar1=-0.5, scalar2=1.5 + EPS,
                    op0=ALU.mult, op1=ALU.add)
                nc.vector.tensor_scalar_max(out=rstd[:], in0=rstd[:],
                                            scalar1=0.08)
                t1 = statsp.tile([128, n], f32, tag="nr_t1")
                for _ in range(3):
                    nc.vector.tensor_mul(t1[:], rstd[:], rstd[:])
                    nc.vector.tensor_tensor(t1[:], t1[:], v[:], ALU.mult)
                    nc.vector.tensor_scalar(
                        out=t1[:], in0=t1[:], scalar1=-0.5,
                        scalar2=1.5 + 0.5 * EPS,
                        op0=ALU.mult, op1=ALU.add)
                    nc.vector.tensor_mul(rstd[:], rstd[:], t1[:])

            # ---------- KV phase (K before the loop; V inside chunk 0) ----
            def kv_tile_mms(wkv_t, ow, t, o0):
                ps = ps_o.tile([128, 512], f32, tag="pso", name="kvpso")[:]
                for kc in range(KC_KV):
                    nc.tensor.matmul(
                        ps[:, :ow],
                        yT_sb[:, kc, t * 128:(t + 1) * 128],
                        wkv_t[:, kc, :ow],
                        start=(kc == 0), stop=False)
                nc.tensor.matmul(
                    ps[:, :ow], ones1[:],
                    bkv_bc[0:1, o0:o0 + ow], start=False, stop=True)
                return ps

            def emit_kv_k():
                kln = qrope2
                for (o0, ow) in k_tiles:
                    # stage the weight tile in an attn-pool buffer (the attn
                    # pool is idle during the KV phase)
                    astage = attnp.tile([128, H, S2], bf16, tag="attn")
                    wkv_t = astage[:].rearrange("p h t -> p (h t)") \
                        [:, 0:KC_KV * 512].rearrange("p (k o) -> p k o", o=512)
                    nc.sync.dma_start(
                        wkv_t[:, :, :ow],
                        wkv[:].rearrange("(k p) o -> p k o", p=128)[:, :, o0:o0 + ow])
                    for t in range(2):
                        ps = kv_tile_mms(wkv_t, ow, t, o0)
                        g0 = o0 // 352 * 4
                        sqk = sqp1.tile([128, 352], f32, tag="sqk")
                        nc.scalar.activation(out=sqk[:], in_=ps[:, :ow],
                                             func=AF.Square)
                        ssqk = statsp.tile([128, 4], f32, tag="ssqk")
                        nc.vector.tensor_reduce(
                            ssqk[:], sqk[:].rearrange("p (g d) -> p g d", d=D),
                            axis=mybir.AxisListType.X, op=ALU.add)
                        rstdk = statsp.tile([128, 4], f32, tag="rstdk")
                        emit_rstd(rstdk, ssqk[:], 4, 1.0 / D)
                        psv = ps[:, :ow].rearrange("p (g d) -> p g d", d=D)
                        for g in range(4):
                            if g % 2 == 0:
                                nc.scalar.activation(
                                    out=kln[t][:, g0 + g, 0:D],
                                    in_=psv[:, g, :], func=AF.Copy,
                                    scale=rstdk[:, g:g + 1])
                            else:
                                nc.vector.tensor_scalar_mul(
                                    out=kln[t][:, g0 + g, 0:D],
                                    in0=psv[:, g, :],
                                    scalar1=rstdk[:, g:g + 1])
                for t in range(2):
                    nc.sync.dma_start_transpose(
                        kT[:, :, t * 128:(t + 1) * 128],
                        kln[t][:].rearrange("p h d -> p (h d)"))

            def emit_kv_v():
                # V projection, emitted inside chunk 0 so the PE has Q-proj
                # work while the big weight DMAs land.
                for i, (o0, ow) in enumerate(v_tiles):
                    astage = attnp.tile([128, H, S2], bf16, tag="attn",
                                        name="vstage")
                    wkv_t = astage[:].rearrange("p h t -> p (h t)") \
                        [:, 0:KC_KV * 512].rearrange("p (k o) -> p k o", o=512)
                    nc.sync.dma_start(
                        wkv_t[:, :, :ow],
                        wkv[:].rearrange("(k p) o -> p k o", p=128)[:, :, o0:o0 + ow])
                    for t in range(2):
                        ps = kv_tile_mms(wkv_t, ow, t, o0)
                        if i % 2 == 0:
                            nc.scalar.copy(v_sb[:, t, o0 - CQ:o0 - CQ + ow],
                                           ps[:, :ow])
                        else:
                            nc.vector.tensor_copy(
                                v_sb[:, t, o0 - CQ:o0 - CQ + ow], ps[:, :ow])

            # ---------- per-chunk stages ----------
            def emit_score_head(st, h):
                if h % 2 == 0:
                    st['sps'] = ps_sc.tile([128, 2, S2], f32, tag="sc", name="sc")
                sps = st['sps'][:, h % 2, :]
                nc.tensor.matmul(sps, st['qropeT'][:, h, :], kT[:, h, :],
                                 start=True, stop=True)
                nc.scalar.activation(
                    out=st['attn'][:, h, :], in_=sps,
                    func=AF.Exp,
                    scale=st['rstd'][:, h:h + 1],
                    accum_out=st['denom'][:, h:h + 1])

            def emit_finish_prev(st):
                # normalize + transpose the previous chunk's attention
                si = st['si']
                rd = statsp.tile([128, H], f32, tag="rd")
                nc.vector.reciprocal(rd[:], st['denom'][:])
                attn = st['attn']
                aT = aTp[(si // 2) % 2]
                for h in range(H):
                    eng = nc.vector if (h % 4) < 3 else nc.gpsimd
                    eng.tensor_scalar_mul(
                        out=attn[:, h, :], in0=attn[:, h, :],
                        scalar1=rd[:, h:h + 1])
                    if h % 4 == 3:
                        h0 = h - 3
                        nc.sync.dma_start_transpose(
                            aT[:, 2 * h0:2 * h0 + 8,
                               (si % 2) * 128:(si % 2) * 128 + 128],
                            attn[:, h0:h0 + 4, :]
                            .rearrange("p h t -> p (h t)"))

            def emit_pv(pair):
                # PV for chunks (pair, pair+1); N=256 via the paired aT tile.
                # ctx stays head-major [d, h, s] in SBUF (no remap DMAs);
                # out-proj contracts per-head 88-partition chunks instead.
                aT = aTp[(pair // 2) % 2]
                cstage = cstagep.tile([D, H, 256], bf16, tag="cstage")
                cps2 = None
                for h in range(H):
                    if h % 2 == 0:
                        cps2 = ps_ctx.tile([D, 2, 256], f32, tag="cps")
                    cps = cps2[:, h % 2, :]
                    for t in range(2):
                        nc.tensor.matmul(
                            cps,
                            v_sb[:, t, h * D:(h + 1) * D],
                            aT[:, 2 * h + t, :],
                            start=(t == 0), stop=(t == 1))
                    nc.vector.tensor_copy(cstage[:, h, :], cps)
                return cstage

            def emit_outproj(pair, cstage):
                for s2 in range(2):
                    sj = pair + s2
                    osb = outsbp.tile([128, CQ], bf16, tag="outsb")
                    for (o0, ow) in OSL:
                        pso = ps_o.tile([128, 512], f32, tag="pso")
                        for c in range(H):
                            nc.tensor.matmul(
                                pso[:, :ow],
                                cstage[:, c, s2 * 128:(s2 + 1) * 128],
                                wout_sb[:, c, o0:o0 + ow],
                                start=(c == 0), stop=False)
                        nc.tensor.matmul(
                            pso[:, :ow], ones1[:],
                            bout_bc[0:1, o0:o0 + ow], start=False, stop=True)
                        nc.vector.tensor_copy(osb[:, o0:o0 + ow], pso[:, :ow])
                    nc.sync.dma_start(
                        out[sj * 128:(sj + 1) * 128, :], osb[:])

            def emit_body():
                prev = None
                pv_ctxT = {}
                loads = {}

                def emit_loads(si):
                    xq = xqp.tile([128, KC_Q, 128], bf16, tag="xq")
                    nc.sync.dma_start(
                        xq[:], xT[:].rearrange("(k p) s -> p k s", p=128)
                        [:, :, si * 128:(si + 1) * 128])
                    cw_sb = csp.tile([128, DP], f32, tag="cw")
                    nc.sync.dma_start(cw_sb[:], cw[si * 128:(si + 1) * 128, :])
                    sw_sb = csp.tile([128, DP], f32, tag="sw")
                    nc.sync.dma_start(sw_sb[:], sw[si * 128:(si + 1) * 128, :])
                    if use_badd:
                        ba_sb = csp.tile([128, DP], f32, tag="ba")
                        nc.sync.dma_start(ba_sb[:],
                                          badd[si * 128:(si + 1) * 128, :])
                        loads[si] = (xq, (cw_sb, sw_sb, ba_sb))
                    else:
                        loads[si] = (xq, (cw_sb, sw_sb))

                emit_loads(0)
                for si in range(NS):
                    if si + 1 < NS:
                        emit_loads(si + 1)
                    xq, sl_cw = loads.pop(si)

                    qrope = qrope2[si % 2]
                    qropeT = qropeTp.tile([DT, H, 128], bf16, tag="qropeT")
                    ssq = statsp.tile([128, H], f32, tag="ssq")
                    rstd = statsp.tile([128, H], f32, tag="rstd")
                    denom = statsp.tile([128, H], f32, tag="denom")

                    qps = ps_q.tile([128, 3, 512], f32, tag="qps")
                    qfl = qps[:].rearrange("p a b -> p (a b)")

                    def emit_variance(f0, h0, nh_):
                        sq = sqp1.tile([128, CQ], bf16, tag="sqq")
                        nc.scalar.activation(
                            out=sq[:, f0:f0 + nh_ * D],
                            in_=qfl[:, f0:f0 + nh_ * D], func=AF.Square)
                        nc.vector.tensor_reduce(
                            ssq[:, h0:h0 + nh_],
                            sq[:, f0:f0 + nh_ * D].rearrange(
                                "p (h d) -> p h d", d=D),
                            axis=mybir.AxisListType.X, op=ALU.add)

                    def consumers(f0, h0, nh_):
                        psv = qfl[:, f0:f0 + nh_ * D].rearrange(
                            "p (h d) -> p h d", d=D)
                        qv = qrope[:, h0:h0 + nh_, 0:D]
                        nc.vector.tensor_tensor(
                            qv, psv,
                            sl_cw[0][:, None, 0:D].to_broadcast([128, nh_, D]),
                            ALU.mult)
                        tt = sqp1.tile([128, H, D], bf16, tag="tt")
                        nc.vector.tensor_tensor(
                            tt[:, h0:h0 + nh_, 0:HALF],
                            psv[:, :, HALF:D],
                            sl_cw[1][:, None, 0:HALF]
                            .to_broadcast([128, nh_, HALF]), ALU.mult)
                        nc.vector.tensor_tensor(
                            tt[:, h0:h0 + nh_, HALF:D],
                            psv[:, :, 0:HALF],
                            sl_cw[1][:, None, HALF:D]
                            .to_broadcast([128, nh_, HALF]), ALU.mult)
                        if use_badd:
                            nc.vector.tensor_tensor(
                                qv, qv,
                                sl_cw[2][:, None, 0:D]
                                .to_broadcast([128, nh_, D]), ALU.add)
                        nc.vector.tensor_tensor(
                            qv, qv, tt[:, h0:h0 + nh_, :], ALU.add)

                    nh = 0   # next interleaved score head for prev chunk

                    # pass A: output slices 0,1
                    for kc in range(KC_Q):
                        for o in range(2):
                            nc.tensor.matmul(
                                qps[:, o, :], xq[:, kc, :],
                                wq_sb[:, kc, o * 512:(o + 1) * 512],
                                start=(kc == 0), stop=(kc == KC_Q - 1))
                        if prev is not None and kc in (3, 5, 7, 9, 11):
                            emit_score_head(prev, nh)
                            emit_score_head(prev, nh + 1)
                            nh += 2

                    if si == 0:
                        emit_kv_k()
                    consumers(0, 0, HA)
                    emit_variance(0, 0, HA)
                    nc.sync.dma_start_transpose(
                        qropeT[:, 0:8, :],
                        qrope[:, 0:8, :].rearrange("p h d -> p (h d)"))

                    # pass B: output slice 2
                    for kc in range(KC_Q):
                        nc.tensor.matmul(
                            qps[:, 2, 0:384], xq[:, kc, :],
                            wq_sb[:, kc, 1024:1408],
                            start=(kc == 0), stop=(kc == KC_Q - 1))
                        if prev is not None and kc in (3, 6, 9):
                            emit_score_head(prev, nh)
                            emit_score_head(prev, nh + 1)
                            nh += 2

                    consumers(FA, HA, HB)
                    if si == 0:
                        emit_kv_v()
                    if si == 1:
                        emit_wout_load()
                    emit_variance(FA, HA, HB)
                    emit_rstd(rstd, ssq[:], H, 1.0 / D)
                    nc.sync.dma_start_transpose(
                        qropeT[:, 8:16, :],
                        qrope[:, 8:16, :].rearrange("p h d -> p (h d)"))

                    attn = attnp.tile([128, H, S2], bf16, tag="attn")
                    cur = dict(si=si, qropeT=qropeT, attn=attn, denom=denom,
                               rstd=rstd)
                    if prev is not None:
                        emit_finish_prev(prev)

                    if si % 2 == 1 and si >= 3:
                        pair = si - 3
                        pv_ctxT[pair] = emit_pv(pair)
                    if si % 2 == 0 and si >= 4:
                        pair = si - 4
                        emit_outproj(pair, pv_ctxT.pop(pair))

                    prev = cur

                # drain: scores+normalize for chunk NS-1, last PV/out-projs
                for h in range(H):
                    emit_score_head(prev, h)
                emit_finish_prev(prev)
                pair = NS - 2
                pv_ctxT[pair] = emit_pv(pair)
                emit_outproj(NS - 4, pv_ctxT.pop(NS - 4))
                emit_outproj(pair, pv_ctxT.pop(pair))

            for _rep in range(reps):
                emit_body()

    nc.finalize()
    return nc


def _prep(inputs):
    """Host-side shared (per-core independent parts built in kernel())."""
    x = np.asarray(inputs['x'], np.float32)
    y = np.asarray(inputs['y'], np.float32)
    cos = np.asarray(inputs['cos'], np.float32)
    sin = np.asarray(inputs['sin'], np.float32)
    Wq = np.asarray(inputs['Wq'], np.float32)
    bq = np.asarray(inputs['bq'], np.float32)
    Wkv = np.asarray(inputs['Wkv'], np.float32)
    bkv = np.asarray(inputs['bkv'], np.float32)
    qn_w = np.asarray(inputs['qn_w'], np.float32)
    qn_b = np.asarray(inputs['qn_b'], np.float32)
    kn_w = np.asarray(inputs['kn_w'], np.float32)
    kn_b = np.asarray(inputs['kn_b'], np.float32)  # noqa: F841  (cancels in softmax)
    Wout = np.asarray(inputs['Wout'], np.float32)
    bout = np.asarray(inputs['bout'], np.float32)

    perm = np.concatenate([np.arange(0, D, 2), np.arange(1, D, 2)])
    swapv = np.concatenate([np.arange(HALF, D), np.arange(0, HALF)])
    sign = np.concatenate([-np.ones(HALF, np.float32), np.ones(HALF, np.float32)])

    # mean-centering projector (per head); LN(q) = P q * rstd with rstd
    # computed from the second moment only
    P = (np.eye(D) - np.ones((D, D), np.float32) / D).astype(np.float32)

    # Q weights: permute head_dim within each head, mean-center, transpose,
    # append bias row
    Wq_p = Wq.reshape(H, D, CQ)[:, perm, :]
    Wq_p = np.einsum('de,hec->hdc', P, Wq_p).reshape(CQ, CQ)
    bq_p = bq.reshape(H, D)[:, perm]
    bq_p = np.einsum('de,he->hd', P, bq_p).reshape(CQ)
    wq_ext = np.zeros((128 * KC_Q, CQ), np.float32)
    wq_ext[:CQ] = Wq_p.T
    wq_ext[CQ] = bq_p

    # KV: permute + mean-center k-half head_dim (bias too), transpose
    Wkv_p = Wkv.reshape(2, H, D, CKV).copy()
    Wkv_p[0] = np.einsum('de,hec->hdc', P, Wkv_p[0][:, perm, :])
    bkv_p = bkv.reshape(2, H, D).copy()
    bkv_p[0] = np.einsum('de,he->hd', P, bkv_p[0][:, perm])
    wkvT = Wkv_p.reshape(2 * CQ, CKV).T.copy()
    bkv_p = bkv_p.reshape(2 * CQ)

    wq_vec = qn_w[perm]
    wk_vec = kn_w[perm]
    bq_ln = qn_b[perm]

    cos_p = cos[:, perm]
    sin_p = sin[:, perm]
    wfold = wq_vec * wk_vec
    scale = float(D) ** -0.5                    # attention scale folded in
    CW = cos_p * wfold[None, :] * scale                            # [S1, D]
    SW = sign[None, :] * sin_p * (wq_vec[swapv] * wk_vec)[None, :] * scale
    use_badd = bool(np.any(bq_ln != 0.0))
    BA = wk_vec[None, :] * (bq_ln[None, :] * cos_p
                            + sign[None, :] * bq_ln[swapv][None, :] * sin_p) * scale

    return dict(
        x=x, y=y, wq_ext=wq_ext, wkvT=wkvT, bkv_p=bkv_p,
        woutT=Wout.T.copy(), bout=bout, CW=CW, SW=SW, BA=BA,
        use_badd=use_badd)


def _make_in_maps(p):
    use_badd = p['use_badd']
    wq_bf = p['wq_ext'].astype(ml_dtypes.bfloat16)
    wkv_bf = p['wkvT'].astype(ml_dtypes.bfloat16)
    wout_bf = p['woutT'].astype(ml_dtypes.bfloat16)
    in_maps = []
    for c in range(NCORES):
        b = c // 2
        s0 = (c % 2) * S
        xTe = np.zeros((128 * KC_Q, S), np.float32)
        xTe[:CQ] = p['x'][b, s0:s0 + S].T
        xTe[CQ] = 1.0
        cwp = np.zeros((S, DP), np.float32)
        cwp[:, :D] = p['CW'][s0:s0 + S]
        swp = np.zeros((S, DP), np.float32)
        swp[:, :D] = p['SW'][s0:s0 + S]
        m = {
            'xT': xTe.astype(ml_dtypes.bfloat16),
            'yT': p['y'][b].T.astype(ml_dtypes.bfloat16).copy(),
            'wq': wq_bf, 'wkv': wkv_bf, 'wout': wout_bf,
            'bkv': p['bkv_p'].astype(ml_dtypes.bfloat16),
            'bout': p['bout'].astype(ml_dtypes.bfloat16),
            'cw': cwp, 'sw': swp,
        }
        if use_badd:
            bap = np.zeros((S, DP), np.float32)
            bap[:, :D] = p['BA'][s0:s0 + S]
            m['badd'] = bap
        in_maps.append(m)
    return in_maps


def get_nc(use_badd, reps=1):
    key = (use_badd, reps)
    if key not in _BUILD_CACHE:
        _BUILD_CACHE[key] = _build(use_badd, reps)
    return _BUILD_CACHE[key]


def kernel(**inputs) -> np.ndarray:
    p = _prep(inputs)
    in_maps = _make_in_maps(p)
    nc = get_nc(p['use_badd'])
    res = run_bass_kernel_spmd(nc, in_maps, core_ids=list(range(NCORES)))
    outp = np.empty((B, S1, CQ), np.float32)
    for c in range(NCORES):
        b = c // 2
        s0 = (c % 2) * S
        outp[b, s0:s0 + S] = np.asarray(res.results[c]['out'], np.float32)
    return outp
